# revision 2
# baseline (speedup 1.0000x reference)
"""Trainium2 Bass kernel for the 3-metalayer forward-forward style MLP.

Distribution: the (10 labels x 512 batch) grid flattens to 5120 independent
rows; each of the 8 cores processes 640 rows (pure data parallelism, weights
replicated, no collectives).

Device-side algorithm (per core, rows R=640):
  - normalized states kept feature-major [2048(part-chunks), R] in fp8e4m3
    scaled x64; weights fp8e4m3 prescaled x64 on host; matmuls run in
    DoubleRow perf mode (2 k-chunks per instruction, 2x PE throughput),
    fp32 PSUM accumulate, ACT relu+bias eviction with scale=1/4096
  - 0.7/0.3 metalayer blend folded into host-prescaled weights/biases
    (relu positive homogeneity)
  - row L2 norms: square (DVE, bf16) + ones-vector PE matmul reduction over
    partitions; 1/(sqrt+eps) on DVE; the M=128 ones-matmul broadcasts the
    row sum-of-squares to every partition for free; goodness = sum(s^2)/2048
    falls out of the same machinery
  - t=0 terms with zero-state inputs are host-folded constants; the layer-1
    "pre" term (static overlay input) is computed once and reused all 3 steps
"""

import numpy as np
import ml_dtypes

import concourse.bass as bass
import concourse.tile as tile
from concourse import bacc, mybir
from concourse.bass_utils import run_bass_kernel_spmd

BF = mybir.dt.bfloat16
F8 = mybir.dt.float8e4
F32 = mybir.dt.float32
NPBF = ml_dtypes.bfloat16
NPF8 = ml_dtypes.float8_e4m3
DR = mybir.MatmulPerfMode.DoubleRow

N_CORES = 8
P = 128
D_IN = 784
D_IN_PAD = 1024           # 8 * 128 (padded so KC1 is even for DoubleRow)
KC1 = 8                   # k-chunks for the 784->2048 matmul
KC = 16                   # k-chunks for 2048-contraction matmuls
MC = 16                   # output-feature chunks (2048 / 128)
H = 2048
B = 512
NL = 10
ROWS = NL * B             # 5120
R = ROWS // N_CORES       # 640 rows per core
RH = 320                  # psum row-chunk (2 per core-row-block)
EPS = 1e-4

SX = 64.0                 # fp8 scale on normalized states
SW = 64.0                 # fp8 scale on weights
SCL = 1.0 / (SX * SW)     # eviction scale undoing both

# bias/const column indices inside the packed [128, 12*16] bias tensor
B1PRE, B1POST, B1SELF, B2PRE, B2POST, B2SELF, B3PRE, B3SELF, C1, C2, C3, C3P = range(12)
NBIAS = 12

_NC_CACHE = {}


def _build_nc():
    """Build the single-core Tile program (same NEFF for all 8 cores)."""
    nc = bacc.Bacc("TRN2", target_bir_lowering=False, debug=False,
                   num_devices=N_CORES)

    hx_d = nc.dram_tensor("hxn", [P, KC1, R], F8, kind="ExternalInput")
    w_d = {
        "w1pre": nc.dram_tensor("w1pre", [MC, P, KC1, P], F8, kind="ExternalInput"),
    }
    for name in ("w1post", "w1self", "w2pre", "w2post", "w2self", "w3pre", "w3self"):
        w_d[name] = nc.dram_tensor(name, [MC, P, KC, P], F8, kind="ExternalInput")
    bias_d = nc.dram_tensor("biases", [P, NBIAS * MC], F32, kind="ExternalInput")
    g_d = nc.dram_tensor("g", [1, R], F32, kind="ExternalOutput")

    with tile.TileContext(nc) as tc:
        with (
            tc.tile_pool(name="consts", bufs=1) as consts,
            tc.tile_pool(name="states", bufs=1) as states,
            tc.tile_pool(name="wpool", bufs=8) as wpool,
            tc.tile_pool(name="epool", bufs=6) as epool,
            tc.tile_pool(name="sqpool", bufs=6) as sqpool,
            tc.tile_pool(name="small", bufs=2) as small,
            tc.tile_pool(name="mmps", bufs=6, space="PSUM") as mmps,
            tc.tile_pool(name="redps", bufs=2, space="PSUM") as redps,
        ):
            # startup order: first hx chunk + first weight block must land
            # before anything else so the PE starts within ~1.5us
            hx = states.tile([P, KC1, R], F8, tag="hxn")
            nc.sync.dma_start(out=hx[:, 0:2, :], in_=hx_d[:, 0:2, :])
            bias_sb = consts.tile([P, NBIAS * MC], F32)
            w0 = wpool.tile([P, KC1, P], F8, tag="w", name="w1pre0")
            nc.sync.dma_start(out=w0[:], in_=w_d["w1pre"][0])
            nc.sync.dma_start(out=bias_sb[:], in_=bias_d[:])
            for kc in range(2, KC1, 2):
                nc.sync.dma_start(out=hx[:, kc:kc + 2, :], in_=hx_d[:, kc:kc + 2, :])
            # [128, 128] ones: M=128 ones-matmul both reduces over partitions
            # AND broadcasts the row sum-of-squares to every partition for free
            ones_red = consts.tile([P, P], BF)
            nc.vector.memset(ones_red[:], 1.0)
            gacc = consts.tile([1, R], F32)

            # warm the PE HAM clock gate while the initial DMAs are in
            # flight: ~25 dummy matmuls span >3.4us of PE activity, so the
            # real matmul stream starts at 2.4GHz instead of 1.2GHz
            warm_ps = mmps.tile([P, RH], F32, tag="mm", name="warm_ps")
            for _ in range(64):
                nc.tensor.matmul(warm_ps[:, :P], ones_red[:], ones_red[:],
                                 start=True, stop=True)
            At = states.tile([P, MC, R], BF, tag="A")
            s1 = states.tile([P, MC, R], F8, tag="s1")
            s2 = states.tile([P, MC, R], F8, tag="s2")
            s3 = states.tile([P, MC, R], F8, tag="s3")
            snew = states.tile([P, MC, R], BF, tag="snew")
            comb = states.tile([P, MC, R], BF, tag="comb")

            _red_uid = [0]

            def red_pair():
                _red_uid[0] += 1
                u = _red_uid[0]
                return (redps.tile([P, RH], F32, tag="red", name=f"red{u}a"),
                        redps.tile([P, RH], F32, tag="red", name=f"red{u}b"))

            def bias_ap(idx, mc):
                col = idx * MC + mc
                return bias_sb[:, col:col + 1]

            def rsl(rh):
                return slice(rh * RH, (rh + 1) * RH)

            def term_pass(wname, kcn, src, evict, w0_tile=None, defer=2):
                """One linear term: stream weight blocks, accumulate psums,
                hand each [128, RH] psum chunk to `evict(mc, rh, ps)`.

                Evictions are emitted `defer` psum-groups late: the eviction
                chain (ACT relu -> DVE combine/square -> PE reduce-matmul)
                has ~1.5us of cross-engine latency, and emitting it inline
                makes the strict-FIFO PE queue stall on the reduce-matmul.
                Deferring places it behind independent matmul work."""
                wd = w_d[wname]
                pending = []
                for mc in range(MC):
                    if mc == 0 and w0_tile is not None:
                        wt = w0_tile
                    else:
                        wt = wpool.tile([P, kcn, P], F8, tag="w")
                        nc.sync.dma_start(out=wt[:], in_=wd[mc])
                    for rh in range(2):
                        ps = mmps.tile([P, RH], F32, tag="mm")
                        for kc in range(0, kcn, 2):
                            nc.tensor.matmul(
                                ps[:], wt[:, kc:kc + 2, :],
                                src[:, kc:kc + 2, rsl(rh)],
                                start=(kc == 0), stop=(kc == kcn - 2),
                                perf_mode=DR)
                        pending.append((mc, rh, ps))
                        if len(pending) > defer:
                            evict(*pending.pop(0))
                while pending:
                    evict(*pending.pop(0))

            def sq_and_reduce(mc, rh, red):
                """Square the freshly written snew chunk; accumulate row
                sum-of-squares into the red psum via a ones-matmul."""
                sq = sqpool.tile([P, RH], BF, tag="sq")
                # on DVE (not ACT): keeps the ACT queue pure relu-evictions,
                # avoiding head-of-line blocking behind the DVE combine
                nc.vector.tensor_mul(sq[:], snew[:, mc, rsl(rh)],
                                     snew[:, mc, rsl(rh)])
                nc.tensor.matmul(red[rh][:], ones_red[:], sq[:],
                                 start=(mc == 0), stop=(mc == MC - 1))

            def finale(red, tgt, goodness):
                """red[rh] holds sum(s^2) per row, already broadcast across
                all 128 partitions (M=128 ones-matmul). sqrt + eps +
                fast-reciprocal scaled so tgt = SX * snew/(norm+eps) lands
                in fp8 range."""
                if goodness:
                    for rh in range(2):
                        if goodness == "init":
                            nc.vector.tensor_copy(gacc[:, rsl(rh)],
                                                  red[rh][0:1, :])
                        else:
                            nc.vector.tensor_add(gacc[:, rsl(rh)],
                                                 gacc[:, rsl(rh)],
                                                 red[rh][0:1, :])
                if tgt is None:
                    return
                nr = small.tile([P, R], F32, tag="nr")
                for rh in range(2):
                    # sqrt(red/SX^2) = sqrt(ss)/SX
                    nc.scalar.activation(nr[:, rsl(rh)], red[rh][:],
                                         mybir.ActivationFunctionType.Sqrt,
                                         scale=1.0 / (SX * SX))
                nc.vector.tensor_scalar_add(nr[:], nr[:], EPS / SX)
                inv = small.tile([P, R], F32, tag="inv")
                nc.vector.reciprocal_approx_fast(out=inv[:], in_=nr[:])
                for rh in range(2):
                    for mc in range(MC):
                        nc.vector.tensor_mul(tgt[:, mc, rsl(rh)],
                                             snew[:, mc, rsl(rh)],
                                             inv[:, rsl(rh)])

            def evict_to(dst, bidx):
                def ev(mc, rh, ps):
                    nc.scalar.activation(
                        dst[:, mc, rsl(rh)], ps[:],
                        mybir.ActivationFunctionType.Relu,
                        bias=bias_ap(bidx, mc), scale=SCL)
                return ev

            def evict_add_comb(bidx):
                def ev(mc, rh, ps):
                    e = epool.tile([P, RH], F32, tag="e")
                    nc.scalar.activation(
                        e[:], ps[:], mybir.ActivationFunctionType.Relu,
                        bias=bias_ap(bidx, mc), scale=SCL)
                    nc.vector.tensor_add(comb[:, mc, rsl(rh)],
                                         e[:], comb[:, mc, rsl(rh)])
                return ev

            # ---- A = relu(hxn @ w1pre' + 0.7*b1pre), cached for all steps.
            # t0-n1 (snew = A + c1) is fused into the same pass so its
            # elementwise work overlaps the A matmuls chunk by chunk.
            red = red_pair()

            def ev_a(mc, rh, ps, red=red):
                nc.scalar.activation(
                    At[:, mc, rsl(rh)], ps[:],
                    mybir.ActivationFunctionType.Relu,
                    bias=bias_ap(B1PRE, mc), scale=SCL)
                nc.vector.tensor_scalar_add(
                    snew[:, mc, rsl(rh)], At[:, mc, rsl(rh)],
                    bias_ap(C1, mc))
                sq_and_reduce(mc, rh, red)

            # defer=4: the A pass produces chunks every ~0.5us (4 dr-chunks),
            # so the ~1.5us eviction chain needs extra slack to stay hidden
            term_pass("w1pre", KC1, hx, ev_a, w0_tile=w0, defer=4)
            finale(red, s1, None)

            # ---- t0, n2 / n3: single pre-term + const.
            # t1-n1's post/self term passes are wedged between them: they
            # only need s2(t0)/s1(t0) and don't touch comb (the t0 updates
            # don't use it), so their matmuls fill t0's serial-chain tails.
            def ev_t0(red, cidx, bpre):
                def ev(mc, rh, ps):
                    e = epool.tile([P, RH], F32, tag="e")
                    nc.scalar.activation(
                        e[:], ps[:], mybir.ActivationFunctionType.Relu,
                        bias=bias_ap(bpre, mc), scale=SCL)
                    nc.vector.tensor_scalar_add(
                        snew[:, mc, rsl(rh)], e[:], bias_ap(cidx, mc))
                    sq_and_reduce(mc, rh, red)
                return ev

            red = red_pair()
            term_pass("w2pre", KC, s1, ev_t0(red, C2, B2PRE))
            finale(red, s2, None)

            term_pass("w1post", KC, s2, evict_to(comb, B1POST))
            term_pass("w1self", KC, s1, evict_add_comb(B1SELF))

            red = red_pair()
            term_pass("w3pre", KC, s2, ev_t0(red, C3, B3PRE))
            finale(red, s3, None)

            def n1_combine(last):
                red = red_pair()
                for mc in range(MC):
                    for rh in range(2):
                        nc.vector.tensor_add(snew[:, mc, rsl(rh)],
                                             At[:, mc, rsl(rh)],
                                             comb[:, mc, rsl(rh)])
                        sq_and_reduce(mc, rh, red)
                finale(red, s1, "init" if last else None)

            # ---- t1 / t2
            for t in (1, 2):
                last = (t == 2)
                # n1 = A + relu(s2@w1post'+b) + relu(s1@w1self'+b)
                if t == 2:
                    term_pass("w1post", KC, s2, evict_to(comb, B1POST))
                    term_pass("w1self", KC, s1, evict_add_comb(B1SELF))
                n1_combine(last)

                # n2 = relu(s1new@w2pre') + relu(s3@w2post') + relu(s2@w2self')
                term_pass("w2post", KC, s3, evict_to(comb, B2POST))
                term_pass("w2self", KC, s2, evict_add_comb(B2SELF))
                red = red_pair()

                def ev_n2(mc, rh, ps, red=red):
                    e = epool.tile([P, RH], F32, tag="e")
                    nc.scalar.activation(
                        e[:], ps[:], mybir.ActivationFunctionType.Relu,
                        bias=bias_ap(B2PRE, mc), scale=SCL)
                    nc.vector.tensor_add(snew[:, mc, rsl(rh)],
                                         e[:], comb[:, mc, rsl(rh)])
                    sq_and_reduce(mc, rh, red)

                term_pass("w2pre", KC, s1, ev_n2)
                finale(red, s2, "add" if last else None)

                # n3 = relu(s2new@w3pre') + c3p + relu(s3@w3self')
                term_pass("w3self", KC, s3, evict_to(comb, B3SELF))
                red = red_pair()

                def ev_n3(mc, rh, ps, red=red):
                    e = epool.tile([P, RH], F32, tag="e")
                    nc.scalar.activation(
                        e[:], ps[:], mybir.ActivationFunctionType.Relu,
                        bias=bias_ap(B3PRE, mc), scale=SCL)
                    nc.vector.scalar_tensor_tensor(
                        snew[:, mc, rsl(rh)], e[:], bias_ap(C3P, mc),
                        comb[:, mc, rsl(rh)],
                        op0=mybir.AluOpType.add, op1=mybir.AluOpType.add)
                    sq_and_reduce(mc, rh, red)

                term_pass("w3pre", KC, s2, ev_n3)
                finale(red, None if last else s3, "add" if last else None)

            # ---- goodness out: g = gacc / 2048
            gout = consts.tile([1, R], F32, tag="gout")
            nc.scalar.mul(gout[:], gacc[:], 1.0 / H)
            nc.sync.dma_start(out=g_d[:], in_=gout[:])

    nc.compile()
    return nc


def _block_weight(w, scale, kcn):
    """[2048, d_in] float32 -> [MC, P, kcn, P] fp8e4m3 blocked for linear DMA:
    host_w[mc, p, kc, m] = scale * SW * W[mc*128+m, kc*128+p]."""
    w = np.asarray(w, dtype=np.float32) * (scale * SW)
    din = w.shape[1]
    if din < kcn * P:
        w = np.pad(w, ((0, 0), (0, kcn * P - din)))
    blk = w.reshape(MC, P, kcn, P).transpose(0, 3, 2, 1)
    return np.ascontiguousarray(blk.astype(NPF8))


def _col(v):
    """[2048] -> [128, 16] (partition-major bias layout)."""
    return np.asarray(v, dtype=np.float32).reshape(MC, P).T


def prepare_inputs(inputs):
    """Host prep: overlay+normalize Hx, prescale/block weights, pack biases.
    Returns (shared_map, per_core_hx list)."""
    x = np.asarray(inputs["x"], dtype=np.float32)
    mx = x.max()
    base = x.copy()
    base[:, :NL] = 0.0
    hx = np.tile(base[None, :, :], (NL, 1, 1))
    for l in range(NL):
        hx[l, :, l] = mx
    hx = hx.reshape(ROWS, D_IN)
    n = np.linalg.norm(hx, axis=1, keepdims=True)
    hxn = hx / (n + EPS) * SX
    hxn = np.pad(hxn, ((0, 0), (0, D_IN_PAD - D_IN)))

    per_core_hx = []
    for c in range(N_CORES):
        h = hxn[c * R:(c + 1) * R].T            # [1024, 640]
        h = h.reshape(KC1, P, R).transpose(1, 0, 2)
        per_core_hx.append(np.ascontiguousarray(h.astype(NPF8)))

    shared = {
        "w1pre": _block_weight(inputs["w1_pre"], 0.7, KC1),
        "w1post": _block_weight(inputs["w1_post"], 0.7, KC),
        "w1self": _block_weight(inputs["w1_self"], 0.3, KC),
        "w2pre": _block_weight(inputs["w2_pre"], 0.7, KC),
        "w2post": _block_weight(inputs["w2_post"], 0.7, KC),
        "w2self": _block_weight(inputs["w2_self"], 0.3, KC),
        "w3pre": _block_weight(inputs["w3_pre"], 0.7, KC),
        "w3self": _block_weight(inputs["w3_self"], 0.3, KC),
    }

    relu = lambda a: np.maximum(np.asarray(a, dtype=np.float32), 0.0)

    cols = np.empty((P, NBIAS * MC), dtype=np.float32)
    vals = {
        B1PRE: 0.7 * np.asarray(inputs["b1_pre"], np.float32),
        B1POST: 0.7 * np.asarray(inputs["b1_post"], np.float32),
        B1SELF: 0.3 * np.asarray(inputs["b1_self"], np.float32),
        B2PRE: 0.7 * np.asarray(inputs["b2_pre"], np.float32),
        B2POST: 0.7 * np.asarray(inputs["b2_post"], np.float32),
        B2SELF: 0.3 * np.asarray(inputs["b2_self"], np.float32),
        B3PRE: 0.7 * np.asarray(inputs["b3_pre"], np.float32),
        B3SELF: 0.3 * np.asarray(inputs["b3_self"], np.float32),
        C1: 0.7 * relu(inputs["b1_post"]) + 0.3 * relu(inputs["b1_self"]),
        C2: 0.7 * relu(inputs["b2_post"]) + 0.3 * relu(inputs["b2_self"]),
        C3: 0.7 * relu(inputs["b3_post"]) + 0.3 * relu(inputs["b3_self"]),
        C3P: 0.7 * relu(inputs["b3_post"]),
    }
    for idx, v in vals.items():
        cols[:, idx * MC:(idx + 1) * MC] = _col(v)
    shared["biases"] = np.ascontiguousarray(cols)

    return shared, per_core_hx


def run(inputs, trace=False):
    shared, per_core_hx = prepare_inputs(inputs)
    if "nc" not in _NC_CACHE:
        _NC_CACHE["nc"] = _build_nc()
    nc = _NC_CACHE["nc"]
    in_maps = [dict(shared, hxn=per_core_hx[c]) for c in range(N_CORES)]
    res = run_bass_kernel_spmd(nc, in_maps, core_ids=list(range(N_CORES)),
                               trace=trace)
    g = np.concatenate([res.results[c]["g"][0] for c in range(N_CORES)])
    out = g.reshape(NL, B).T.astype(np.float32)
    return np.ascontiguousarray(out), res


def kernel(**inputs):
    out, _ = run(inputs, trace=False)
    return out


# revision 12
# speedup vs baseline: 1.0061x; 1.0061x over previous
"""Trainium2 Bass kernel for the 3-metalayer forward-forward style MLP.

Distribution: the (10 labels x 512 batch) grid flattens to 5120 independent
rows; each of the 8 cores processes 640 rows (pure data parallelism, weights
replicated, no collectives).

Device-side algorithm (per core, rows R=640):
  - normalized states kept feature-major [2048(part-chunks), R] in fp8e4m3
    scaled x64; weights fp8e4m3 prescaled x64 on host; matmuls run in
    DoubleRow perf mode (2 k-chunks per instruction, 2x PE throughput),
    fp32 PSUM accumulate, ACT relu+bias eviction with scale=1/4096
  - 0.7/0.3 metalayer blend folded into host-prescaled weights/biases
    (relu positive homogeneity)
  - row L2 norms: square (DVE, bf16) + ones-vector PE matmul reduction over
    partitions; 1/(sqrt+eps) on DVE; the M=128 ones-matmul broadcasts the
    row sum-of-squares to every partition for free; goodness = sum(s^2)/2048
    falls out of the same machinery
  - t=0 terms with zero-state inputs are host-folded constants; the layer-1
    "pre" term (static overlay input) is computed once and reused all 3 steps
"""

import numpy as np
import ml_dtypes

import concourse.bass as bass
import concourse.tile as tile
from concourse import bacc, mybir
from concourse.bass_utils import run_bass_kernel_spmd

BF = mybir.dt.bfloat16
F8 = mybir.dt.float8e4
F32 = mybir.dt.float32
NPBF = ml_dtypes.bfloat16
NPF8 = ml_dtypes.float8_e4m3
DR = mybir.MatmulPerfMode.DoubleRow

N_CORES = 8
P = 128
D_IN = 784
D_IN_PAD = 1024           # 8 * 128 (padded so KC1 is even for DoubleRow)
KC1 = 8                   # k-chunks for the 784->2048 matmul
KC = 16                   # k-chunks for 2048-contraction matmuls
MC = 16                   # output-feature chunks (2048 / 128)
H = 2048
B = 512
NL = 10
ROWS = NL * B             # 5120
R = ROWS // N_CORES       # 640 rows per core
RH = 320                  # psum row-chunk (2 per core-row-block)
EPS = 1e-4

SX = 64.0                 # fp8 scale on normalized states
SW = 64.0                 # fp8 scale on weights
AL = 16.0                 # alpha: unnormalized bf16 states kept x16 so their
                          # squares (256 s^2) sit in fp8e4m3's normal range
SCL = AL / (SX * SW)      # eviction scale undoing fp8 scales, applying alpha

# bias/const column indices inside the packed [128, 12*16] bias tensor
B1PRE, B1POST, B1SELF, B2PRE, B2POST, B2SELF, B3PRE, B3SELF, C1, C2, C3, C3P = range(12)
NBIAS = 12

_NC_CACHE = {}


def _build_nc():
    """Build the single-core Tile program (same NEFF for all 8 cores)."""
    nc = bacc.Bacc("TRN2", target_bir_lowering=False, debug=False,
                   num_devices=N_CORES)

    hx_d = nc.dram_tensor("hxn", [P, KC1, R], F8, kind="ExternalInput")
    w_d = {
        "w1pre": nc.dram_tensor("w1pre", [MC, P, KC1, P], F8, kind="ExternalInput"),
    }
    for name in ("w1post", "w1self", "w2pre", "w2post", "w2self", "w3pre", "w3self"):
        w_d[name] = nc.dram_tensor(name, [MC, P, KC, P], F8, kind="ExternalInput")
    bias_d = nc.dram_tensor("biases", [P, NBIAS * MC], F32, kind="ExternalInput")
    g_d = nc.dram_tensor("g", [1, R], F32, kind="ExternalOutput")

    with tile.TileContext(nc) as tc:
        with (
            tc.tile_pool(name="consts", bufs=1) as consts,
            tc.tile_pool(name="states", bufs=1) as states,
            tc.tile_pool(name="wpool", bufs=8) as wpool,
            tc.tile_pool(name="epool", bufs=6) as epool,
            tc.tile_pool(name="sqpool", bufs=6) as sqpool,
            tc.tile_pool(name="small", bufs=2) as small,
            tc.tile_pool(name="mmps", bufs=6, space="PSUM") as mmps,
            tc.tile_pool(name="redps", bufs=2, space="PSUM") as redps,
        ):
            # startup order: first hx chunk + first weight block must land
            # before anything else so the PE starts within ~1.5us
            hx = states.tile([P, KC1, R], F8, tag="hxn")
            nc.sync.dma_start(out=hx[:, 0:2, :], in_=hx_d[:, 0:2, :])
            bias_sb = consts.tile([P, NBIAS * MC], F32)
            w0 = wpool.tile([P, KC1, P], F8, tag="w", name="w1pre0")
            nc.sync.dma_start(out=w0[:], in_=w_d["w1pre"][0])
            nc.sync.dma_start(out=bias_sb[:], in_=bias_d[:])
            for kc in range(2, KC1, 2):
                nc.sync.dma_start(out=hx[:, kc:kc + 2, :], in_=hx_d[:, kc:kc + 2, :])
            # [128, 2, 128] fp8 ones: M=128 DoubleRow ones-matmul reduces two
            # feature chunks over partitions at once AND broadcasts the row
            # sum-of-squares to every partition for free
            ones_red = consts.tile([P, P], BF)
            nc.vector.memset(ones_red[:], 1.0)
            ones8 = consts.tile([P, 2, P], F8)
            nc.vector.memset(ones8[:], 1.0)
            gacc = consts.tile([1, R], F32)

            # warm the PE HAM clock gate while the initial DMAs are in
            # flight: ~25 dummy matmuls span >3.4us of PE activity, so the
            # real matmul stream starts at 2.4GHz instead of 1.2GHz
            warm_ps = mmps.tile([P, RH], F32, tag="mm", name="warm_ps")
            for _ in range(64):
                nc.tensor.matmul(warm_ps[:, :P], ones_red[:], ones_red[:],
                                 start=True, stop=True)
            At = states.tile([P, MC, R], BF, tag="A")
            s1 = states.tile([P, MC, R], F8, tag="s1")
            s2 = states.tile([P, MC, R], F8, tag="s2")
            s3 = states.tile([P, MC, R], F8, tag="s3")
            snew = states.tile([P, MC, R], BF, tag="snew")
            comb = states.tile([P, MC, R], BF, tag="comb")

            _red_uid = [0]

            def red_pair():
                _red_uid[0] += 1
                u = _red_uid[0]
                return (redps.tile([P, RH], F32, tag="red", name=f"red{u}a"),
                        redps.tile([P, RH], F32, tag="red", name=f"red{u}b"))

            def bias_ap(idx, mc):
                col = idx * MC + mc
                return bias_sb[:, col:col + 1]

            def rsl(rh):
                return slice(rh * RH, (rh + 1) * RH)

            def term_pass(wname, kcn, src, evict, w0_tile=None, defer=2,
                          pre_chunk=None):
                """One linear term: stream weight blocks, accumulate psums,
                hand each [128, RH] psum chunk to `evict(mc, rh, ps)`.

                Evictions are emitted `defer` psum-groups late: the eviction
                chain (ACT relu -> DVE combine/square -> PE reduce-matmul)
                has ~1.5us of cross-engine latency, and emitting it inline
                makes the strict-FIFO PE queue stall on the reduce-matmul.
                Deferring places it behind independent matmul work."""
                wd = w_d[wname]
                pending = []
                for mc in range(MC):
                    if pre_chunk is not None:
                        pre_chunk(mc)
                    if mc == 0 and w0_tile is not None:
                        wt = w0_tile
                    else:
                        wt = wpool.tile([P, kcn, P], F8, tag="w")
                        nc.sync.dma_start(out=wt[:], in_=wd[mc])
                    for rh in range(2):
                        ps = mmps.tile([P, RH], F32, tag="mm")
                        for kc in range(0, kcn, 2):
                            nc.tensor.matmul(
                                ps[:], wt[:, kc:kc + 2, :],
                                src[:, kc:kc + 2, rsl(rh)],
                                start=(kc == 0), stop=(kc == kcn - 2),
                                perf_mode=DR)
                        pending.append((mc, rh, ps))
                        if len(pending) > defer:
                            evict(*pending.pop(0))
                while pending:
                    evict(*pending.pop(0))

            _sq_pairs = {}

            def sq_and_reduce(mc, rh, red):
                """Square the freshly written snew chunk (fp8, 256 s^2);
                every second chunk, accumulate two chunks' row sum-of-squares
                into the red psum via a DoubleRow ones-matmul."""
                if mc % 2 == 0:
                    _sq_pairs[rh] = sqpool.tile([P, 2, RH], F8, tag="sq",
                                                name=f"sqpair{rh}")
                sqt = _sq_pairs[rh]
                # on DVE (not ACT): keeps the ACT queue pure relu-evictions,
                # avoiding head-of-line blocking behind the DVE combine
                nc.vector.tensor_mul(sqt[:, mc % 2, :], snew[:, mc, rsl(rh)],
                                     snew[:, mc, rsl(rh)])
                if mc % 2 == 1:
                    nc.tensor.matmul(red[rh][:], ones8[:], sqt[:],
                                     start=(mc == 1), stop=(mc == MC - 1),
                                     perf_mode=DR)

            def finale(red, tgt, goodness):
                """red[rh] holds sum(s^2) per row, already broadcast across
                all 128 partitions (M=128 ones-matmul). sqrt + eps +
                fast-reciprocal scaled so tgt = SX * snew/(norm+eps) lands
                in fp8 range."""
                if goodness:
                    for rh in range(2):
                        if goodness == "init":
                            nc.vector.tensor_copy(gacc[:, rsl(rh)],
                                                  red[rh][0:1, :])
                        else:
                            nc.vector.tensor_add(gacc[:, rsl(rh)],
                                                 gacc[:, rsl(rh)],
                                                 red[rh][0:1, :])
                if tgt is None:
                    return
                # per-rh chains: rh0's normalize completes (and unblocks the
                # next pass's rh0 matmuls) one DVE-chain earlier than rh1
                nr = small.tile([P, R], F32, tag="nr")
                inv = small.tile([P, R], F32, tag="inv")
                for rh in range(2):
                    # sqrt(red/SX^2) = AL*sqrt(ss)/SX
                    nc.scalar.activation(nr[:, rsl(rh)], red[rh][:],
                                         mybir.ActivationFunctionType.Sqrt,
                                         scale=1.0 / (SX * SX))
                    nc.vector.tensor_scalar_add(nr[:, rsl(rh)],
                                                nr[:, rsl(rh)], AL * EPS / SX)
                    nc.vector.reciprocal_approx_fast(out=inv[:, rsl(rh)],
                                                     in_=nr[:, rsl(rh)])
                    for mc in range(MC):
                        nc.vector.tensor_mul(tgt[:, mc, rsl(rh)],
                                             snew[:, mc, rsl(rh)],
                                             inv[:, rsl(rh)])

            def evict_to(dst, bidx):
                def ev(mc, rh, ps):
                    nc.scalar.activation(
                        dst[:, mc, rsl(rh)], ps[:],
                        mybir.ActivationFunctionType.Relu,
                        bias=bias_ap(bidx, mc), scale=SCL)
                return ev

            def evict_add_comb(bidx):
                def ev(mc, rh, ps):
                    e = epool.tile([P, RH], F32, tag="e")
                    nc.scalar.activation(
                        e[:], ps[:], mybir.ActivationFunctionType.Relu,
                        bias=bias_ap(bidx, mc), scale=SCL)
                    nc.vector.tensor_add(comb[:, mc, rsl(rh)],
                                         e[:], comb[:, mc, rsl(rh)])
                return ev

            # ---- A = relu(hxn @ w1pre' + 0.7*b1pre), cached for all steps.
            # t0-n1 (snew = A + c1) is fused into the same pass so its
            # elementwise work overlaps the A matmuls chunk by chunk.
            red = red_pair()

            def ev_a(mc, rh, ps, red=red):
                nc.scalar.activation(
                    At[:, mc, rsl(rh)], ps[:],
                    mybir.ActivationFunctionType.Relu,
                    bias=bias_ap(B1PRE, mc), scale=SCL)
                nc.vector.tensor_scalar_add(
                    snew[:, mc, rsl(rh)], At[:, mc, rsl(rh)],
                    bias_ap(C1, mc))
                sq_and_reduce(mc, rh, red)

            # defer=4: the A pass produces chunks every ~0.5us (4 dr-chunks),
            # so the ~1.5us eviction chain needs extra slack to stay hidden
            term_pass("w1pre", KC1, hx, ev_a, w0_tile=w0, defer=4)
            finale(red, s1, None)

            # ---- t0, n2 / n3: single pre-term + const.
            # t1-n1's post/self term passes are wedged between them: they
            # only need s2(t0)/s1(t0) and don't touch comb (the t0 updates
            # don't use it), so their matmuls fill t0's serial-chain tails.
            def ev_t0(red, cidx, bpre):
                def ev(mc, rh, ps):
                    e = epool.tile([P, RH], F32, tag="e")
                    nc.scalar.activation(
                        e[:], ps[:], mybir.ActivationFunctionType.Relu,
                        bias=bias_ap(bpre, mc), scale=SCL)
                    nc.vector.tensor_scalar_add(
                        snew[:, mc, rsl(rh)], e[:], bias_ap(cidx, mc))
                    sq_and_reduce(mc, rh, red)
                return ev

            red = red_pair()
            term_pass("w2pre", KC, s1, ev_t0(red, C2, B2PRE))
            finale(red, s2, None)

            term_pass("w1post", KC, s2, evict_to(comb, B1POST))
            term_pass("w1self", KC, s1, evict_add_comb(B1SELF))

            red = red_pair()
            term_pass("w3pre", KC, s2, ev_t0(red, C3, B3PRE))
            finale(red, s3, None)

            def n1_chunk(red):
                """Per-mc hook: n1 = A + comb (DVE adds + sq/reduce),
                interleaved into the following pass's matmul stream so the
                PE never drains while DVE churns through the combine."""
                def hook(mc):
                    for rh in range(2):
                        nc.vector.tensor_add(snew[:, mc, rsl(rh)],
                                             At[:, mc, rsl(rh)],
                                             comb[:, mc, rsl(rh)])
                        sq_and_reduce(mc, rh, red)
                return hook

            # ---- t1 / t2
            for t in (1, 2):
                last = (t == 2)
                # n1 = A + relu(s2@w1post'+b) + relu(s1@w1self'+b)
                if t == 2:
                    term_pass("w1post", KC, s2, evict_to(comb, B1POST))
                    term_pass("w1self", KC, s1, evict_add_comb(B1SELF))

                # n2 = relu(s1new@w2pre') + relu(s3@w2post') + relu(s2@w2self')
                # n1's combine rides inside the w2post pass: chunk mc of the
                # combine is emitted just before w2post's mc matmul block
                # (the w2post eviction's comb[mc] write lands after the
                # combine's read, chunk by chunk)
                red_n1 = red_pair()
                term_pass("w2post", KC, s3, evict_to(comb, B2POST),
                          pre_chunk=n1_chunk(red_n1))
                finale(red_n1, s1, "init" if last else None)
                term_pass("w2self", KC, s2, evict_add_comb(B2SELF))
                red = red_pair()

                def ev_n2(mc, rh, ps, red=red):
                    e = epool.tile([P, RH], F32, tag="e")
                    nc.scalar.activation(
                        e[:], ps[:], mybir.ActivationFunctionType.Relu,
                        bias=bias_ap(B2PRE, mc), scale=SCL)
                    nc.vector.tensor_add(snew[:, mc, rsl(rh)],
                                         e[:], comb[:, mc, rsl(rh)])
                    sq_and_reduce(mc, rh, red)

                term_pass("w2pre", KC, s1, ev_n2)
                finale(red, s2, "add" if last else None)

                # n3 = relu(s2new@w3pre') + c3p + relu(s3@w3self')
                term_pass("w3self", KC, s3, evict_to(comb, B3SELF))
                red = red_pair()

                def ev_n3(mc, rh, ps, red=red):
                    e = epool.tile([P, RH], F32, tag="e")
                    nc.scalar.activation(
                        e[:], ps[:], mybir.ActivationFunctionType.Relu,
                        bias=bias_ap(B3PRE, mc), scale=SCL)
                    nc.vector.scalar_tensor_tensor(
                        snew[:, mc, rsl(rh)], e[:], bias_ap(C3P, mc),
                        comb[:, mc, rsl(rh)],
                        op0=mybir.AluOpType.add, op1=mybir.AluOpType.add)
                    sq_and_reduce(mc, rh, red)

                term_pass("w3pre", KC, s2, ev_n3)
                finale(red, None if last else s3, "add" if last else None)

            # ---- goodness out: g = gacc / (alpha^2 * 2048)
            gout = consts.tile([1, R], F32, tag="gout")
            nc.scalar.mul(gout[:], gacc[:], 1.0 / (AL * AL * H))
            nc.sync.dma_start(out=g_d[:], in_=gout[:])

    nc.compile()
    return nc


def _block_weight(w, scale, kcn):
    """[2048, d_in] float32 -> [MC, P, kcn, P] fp8e4m3 blocked for linear DMA:
    host_w[mc, p, kc, m] = scale * SW * W[mc*128+m, kc*128+p]."""
    w = np.asarray(w, dtype=np.float32) * (scale * SW)
    din = w.shape[1]
    if din < kcn * P:
        w = np.pad(w, ((0, 0), (0, kcn * P - din)))
    blk = w.reshape(MC, P, kcn, P).transpose(0, 3, 2, 1)
    return np.ascontiguousarray(blk.astype(NPF8))


def _col(v):
    """[2048] -> [128, 16] (partition-major bias layout)."""
    return np.asarray(v, dtype=np.float32).reshape(MC, P).T


def prepare_inputs(inputs):
    """Host prep: overlay+normalize Hx, prescale/block weights, pack biases.
    Returns (shared_map, per_core_hx list)."""
    x = np.asarray(inputs["x"], dtype=np.float32)
    mx = x.max()
    base = x.copy()
    base[:, :NL] = 0.0
    hx = np.tile(base[None, :, :], (NL, 1, 1))
    for l in range(NL):
        hx[l, :, l] = mx
    hx = hx.reshape(ROWS, D_IN)
    n = np.linalg.norm(hx, axis=1, keepdims=True)
    hxn = hx / (n + EPS) * SX
    hxn = np.pad(hxn, ((0, 0), (0, D_IN_PAD - D_IN)))

    per_core_hx = []
    for c in range(N_CORES):
        h = hxn[c * R:(c + 1) * R].T            # [1024, 640]
        h = h.reshape(KC1, P, R).transpose(1, 0, 2)
        per_core_hx.append(np.ascontiguousarray(h.astype(NPF8)))

    shared = {
        "w1pre": _block_weight(inputs["w1_pre"], 0.7, KC1),
        "w1post": _block_weight(inputs["w1_post"], 0.7, KC),
        "w1self": _block_weight(inputs["w1_self"], 0.3, KC),
        "w2pre": _block_weight(inputs["w2_pre"], 0.7, KC),
        "w2post": _block_weight(inputs["w2_post"], 0.7, KC),
        "w2self": _block_weight(inputs["w2_self"], 0.3, KC),
        "w3pre": _block_weight(inputs["w3_pre"], 0.7, KC),
        "w3self": _block_weight(inputs["w3_self"], 0.3, KC),
    }

    relu = lambda a: np.maximum(np.asarray(a, dtype=np.float32), 0.0)

    cols = np.empty((P, NBIAS * MC), dtype=np.float32)
    # all bias/const columns carry the alpha=16 state scaling (relu positive
    # homogeneity: AL*relu(u+b) = relu(AL*u + AL*b))
    vals = {
        B1PRE: AL * 0.7 * np.asarray(inputs["b1_pre"], np.float32),
        B1POST: AL * 0.7 * np.asarray(inputs["b1_post"], np.float32),
        B1SELF: AL * 0.3 * np.asarray(inputs["b1_self"], np.float32),
        B2PRE: AL * 0.7 * np.asarray(inputs["b2_pre"], np.float32),
        B2POST: AL * 0.7 * np.asarray(inputs["b2_post"], np.float32),
        B2SELF: AL * 0.3 * np.asarray(inputs["b2_self"], np.float32),
        B3PRE: AL * 0.7 * np.asarray(inputs["b3_pre"], np.float32),
        B3SELF: AL * 0.3 * np.asarray(inputs["b3_self"], np.float32),
        C1: AL * (0.7 * relu(inputs["b1_post"]) + 0.3 * relu(inputs["b1_self"])),
        C2: AL * (0.7 * relu(inputs["b2_post"]) + 0.3 * relu(inputs["b2_self"])),
        C3: AL * (0.7 * relu(inputs["b3_post"]) + 0.3 * relu(inputs["b3_self"])),
        C3P: AL * 0.7 * relu(inputs["b3_post"]),
    }
    for idx, v in vals.items():
        cols[:, idx * MC:(idx + 1) * MC] = _col(v)
    shared["biases"] = np.ascontiguousarray(cols)

    return shared, per_core_hx


def run(inputs, trace=False):
    shared, per_core_hx = prepare_inputs(inputs)
    if "nc" not in _NC_CACHE:
        _NC_CACHE["nc"] = _build_nc()
    nc = _NC_CACHE["nc"]
    in_maps = [dict(shared, hxn=per_core_hx[c]) for c in range(N_CORES)]
    res = run_bass_kernel_spmd(nc, in_maps, core_ids=list(range(N_CORES)),
                               trace=trace)
    g = np.concatenate([res.results[c]["g"][0] for c in range(N_CORES)])
    out = g.reshape(NL, B).T.astype(np.float32)
    return np.ascontiguousarray(out), res


def kernel(**inputs):
    out, _ = run(inputs, trace=False)
    return out


# revision 15
# speedup vs baseline: 1.0188x; 1.0126x over previous
"""Trainium2 Bass kernel for the 3-metalayer forward-forward style MLP.

Distribution: the (10 labels x 512 batch) grid flattens to 5120 independent
rows; each of the 8 cores processes 640 rows (pure data parallelism, weights
replicated, no collectives).

Device-side algorithm (per core, rows R=640):
  - normalized states kept feature-major [2048(part-chunks), R] in fp8e4m3
    scaled x64; weights fp8e4m3 prescaled x64 on host; matmuls run in
    DoubleRow perf mode (2 k-chunks per instruction, 2x PE throughput),
    fp32 PSUM accumulate, ACT relu+bias eviction with scale=1/4096
  - 0.7/0.3 metalayer blend folded into host-prescaled weights/biases
    (relu positive homogeneity)
  - row L2 norms: square (DVE, bf16) + ones-vector PE matmul reduction over
    partitions; 1/(sqrt+eps) on DVE; the M=128 ones-matmul broadcasts the
    row sum-of-squares to every partition for free; goodness = sum(s^2)/2048
    falls out of the same machinery
  - t=0 terms with zero-state inputs are host-folded constants; the layer-1
    "pre" term (static overlay input) is computed once and reused all 3 steps
"""

import numpy as np
import ml_dtypes

import concourse.bass as bass
import concourse.tile as tile
from concourse import bacc, mybir
from concourse.bass_utils import run_bass_kernel_spmd

BF = mybir.dt.bfloat16
F8 = mybir.dt.float8e4
F32 = mybir.dt.float32
NPBF = ml_dtypes.bfloat16
NPF8 = ml_dtypes.float8_e4m3
DR = mybir.MatmulPerfMode.DoubleRow

N_CORES = 8
P = 128
D_IN = 784
D_IN_PAD = 1024           # 8 * 128 (padded so KC1 is even for DoubleRow)
KC1 = 8                   # k-chunks for the 784->2048 matmul
KC = 16                   # k-chunks for 2048-contraction matmuls
MC = 16                   # output-feature chunks (2048 / 128)
H = 2048
B = 512
NL = 10
ROWS = NL * B             # 5120
R = ROWS // N_CORES       # 640 rows per core
RH = 320                  # psum row-chunk (2 per core-row-block)
EPS = 1e-4

SX = 64.0                 # fp8 scale on normalized states
SW = 64.0                 # fp8 scale on weights
AL = 16.0                 # alpha: unnormalized bf16 states kept x16 so their
                          # squares (256 s^2) sit in fp8e4m3's normal range
SCL = AL / (SX * SW)      # eviction scale undoing fp8 scales, applying alpha

# bias/const column indices inside the packed [128, 12*16] bias tensor
B1PRE, B1POST, B1SELF, B2PRE, B2POST, B2SELF, B3PRE, B3SELF, C1, C2, C3, C3P = range(12)
NBIAS = 12

_NC_CACHE = {}


def _build_nc():
    """Build the single-core Tile program (same NEFF for all 8 cores)."""
    nc = bacc.Bacc("TRN2", target_bir_lowering=False, debug=False,
                   num_devices=N_CORES)

    hx_d = nc.dram_tensor("hxn", [P, KC1, R], F8, kind="ExternalInput")
    w_d = {
        "w1pre": nc.dram_tensor("w1pre", [MC, P, KC1, P], F8, kind="ExternalInput"),
    }
    for name in ("w1post", "w1self", "w2pre", "w2post", "w2self", "w3pre", "w3self"):
        w_d[name] = nc.dram_tensor(name, [MC, P, KC, P], F8, kind="ExternalInput")
    bias_d = nc.dram_tensor("biases", [P, NBIAS * MC], F32, kind="ExternalInput")
    g_d = nc.dram_tensor("g", [1, R], F32, kind="ExternalOutput")

    with tile.TileContext(nc) as tc:
        with (
            tc.tile_pool(name="consts", bufs=1) as consts,
            tc.tile_pool(name="states", bufs=1) as states,
            tc.tile_pool(name="wpool", bufs=8) as wpool,
            tc.tile_pool(name="epool", bufs=6) as epool,
            tc.tile_pool(name="sqpool", bufs=6) as sqpool,
            tc.tile_pool(name="small", bufs=2) as small,
            tc.tile_pool(name="mmps", bufs=6, space="PSUM") as mmps,
            tc.tile_pool(name="redps", bufs=2, space="PSUM") as redps,
        ):
            # startup order: first hx chunk + first weight block must land
            # before anything else so the PE starts within ~1.5us
            hx = states.tile([P, KC1, R], F8, tag="hxn")
            nc.sync.dma_start(out=hx[:, 0:2, :], in_=hx_d[:, 0:2, :])
            bias_sb = consts.tile([P, NBIAS * MC], F32)
            w0 = wpool.tile([P, KC1, P], F8, tag="w", name="w1pre0")
            nc.sync.dma_start(out=w0[:], in_=w_d["w1pre"][0])
            nc.sync.dma_start(out=bias_sb[:], in_=bias_d[:])
            for kc in range(2, KC1, 2):
                nc.sync.dma_start(out=hx[:, kc:kc + 2, :], in_=hx_d[:, kc:kc + 2, :])
            # [128, 2, 128] fp8 ones: M=128 DoubleRow ones-matmul reduces two
            # feature chunks over partitions at once AND broadcasts the row
            # sum-of-squares to every partition for free
            ones_red = consts.tile([P, P], BF)
            nc.vector.memset(ones_red[:], 1.0)
            ones8 = consts.tile([P, 2, P], F8)
            nc.vector.memset(ones8[:], 1.0)
            gacc = consts.tile([1, R], F32)

            # warm the PE HAM clock gate while the initial DMAs are in
            # flight: ~25 dummy matmuls span >3.4us of PE activity, so the
            # real matmul stream starts at 2.4GHz instead of 1.2GHz
            warm_ps = mmps.tile([P, RH], F32, tag="mm", name="warm_ps")
            for _ in range(64):
                nc.tensor.matmul(warm_ps[:, :P], ones_red[:], ones_red[:],
                                 start=True, stop=True)
            At = states.tile([P, MC, R], BF, tag="A")
            s1 = states.tile([P, MC, R], F8, tag="s1")
            s2 = states.tile([P, MC, R], F8, tag="s2")
            s3 = states.tile([P, MC, R], F8, tag="s3")
            snew = states.tile([P, MC, R], BF, tag="snew")
            comb = states.tile([P, MC, R], BF, tag="comb")

            _red_uid = [0]

            def red_pair():
                _red_uid[0] += 1
                u = _red_uid[0]
                return (redps.tile([P, RH], F32, tag="red", name=f"red{u}a"),
                        redps.tile([P, RH], F32, tag="red", name=f"red{u}b"))

            def bias_ap(idx, mc):
                col = idx * MC + mc
                return bias_sb[:, col:col + 1]

            def rsl(rh):
                return slice(rh * RH, (rh + 1) * RH)

            def term_pass(wname, kcn, src, evict, w0_tile=None, defer=2,
                          pre_chunk=None):
                """One linear term: stream weight blocks, accumulate psums,
                hand each [128, RH] psum chunk to `evict(mc, rh, ps)`.

                Evictions are emitted `defer` psum-groups late: the eviction
                chain (ACT relu -> DVE combine/square -> PE reduce-matmul)
                has ~1.5us of cross-engine latency, and emitting it inline
                makes the strict-FIFO PE queue stall on the reduce-matmul.
                Deferring places it behind independent matmul work."""
                wd = w_d[wname]
                pending = []
                for mc in range(MC):
                    if pre_chunk is not None:
                        pre_chunk(mc)
                    if mc == 0 and w0_tile is not None:
                        wt = w0_tile
                    else:
                        wt = wpool.tile([P, kcn, P], F8, tag="w")
                        nc.sync.dma_start(out=wt[:], in_=wd[mc])
                    for rh in range(2):
                        ps = mmps.tile([P, RH], F32, tag="mm")
                        for kc in range(0, kcn, 2):
                            nc.tensor.matmul(
                                ps[:], wt[:, kc:kc + 2, :],
                                src[:, kc:kc + 2, rsl(rh)],
                                start=(kc == 0), stop=(kc == kcn - 2),
                                perf_mode=DR)
                        pending.append((mc, rh, ps))
                        if len(pending) > defer:
                            evict(*pending.pop(0))
                while pending:
                    evict(*pending.pop(0))

            _sq_pairs = [None]

            def sq_and_reduce(mc, rh, red):
                """After both rh chunks of snew[mc] are written: square the
                full 640-wide chunk (fp8, 256 s^2) on the otherwise-idle
                GpSimd engine; every second chunk, accumulate two chunks'
                row sum-of-squares into the red psums via DoubleRow
                ones-matmuls."""
                if rh == 0:
                    return
                if mc % 2 == 0:
                    _sq_pairs[0] = sqpool.tile([P, 2, R], F8, tag="sq",
                                               name="sqpair")
                sqt = _sq_pairs[0]
                nc.gpsimd.tensor_mul(sqt[:, mc % 2, :], snew[:, mc, :],
                                     snew[:, mc, :])
                if mc % 2 == 1:
                    for r in range(2):
                        nc.tensor.matmul(red[r][:], ones8[:],
                                         sqt[:, 0:2, rsl(r)],
                                         start=(mc == 1), stop=(mc == MC - 1),
                                         perf_mode=DR)

            def finale(red, tgt, goodness):
                """red[rh] holds sum(s^2) per row, already broadcast across
                all 128 partitions (M=128 ones-matmul). sqrt + eps +
                fast-reciprocal scaled so tgt = SX * snew/(norm+eps) lands
                in fp8 range."""
                if goodness:
                    for rh in range(2):
                        if goodness == "init":
                            nc.vector.tensor_copy(gacc[:, rsl(rh)],
                                                  red[rh][0:1, :])
                        else:
                            nc.vector.tensor_add(gacc[:, rsl(rh)],
                                                 gacc[:, rsl(rh)],
                                                 red[rh][0:1, :])
                if tgt is None:
                    return
                # eps is dropped from 1/(sqrt(ss)+eps): biases guarantee
                # ss >~ 0.1 per row, so the eps term is a <3e-4 relative
                # perturbation -- far below the fp8 noise floor -- and
                # skipping it removes a DVE op from the serial chain
                nr = small.tile([P, R], F32, tag="nr")
                inv = small.tile([P, R], F32, tag="inv")
                for rh in range(2):
                    # sqrt(red/SX^2) = AL*sqrt(ss)/SX
                    nc.scalar.activation(nr[:, rsl(rh)], red[rh][:],
                                         mybir.ActivationFunctionType.Sqrt,
                                         scale=1.0 / (SX * SX))
                nc.vector.reciprocal_approx_fast(out=inv[:], in_=nr[:])
                # 640-wide muls, mc-ascending: the next pass's k-pair DR
                # matmuls unblock two muls at a time
                for mc in range(MC):
                    nc.vector.tensor_mul(tgt[:, mc, :], snew[:, mc, :],
                                         inv[:])

            def evict_to(dst, bidx):
                def ev(mc, rh, ps):
                    nc.scalar.activation(
                        dst[:, mc, rsl(rh)], ps[:],
                        mybir.ActivationFunctionType.Relu,
                        bias=bias_ap(bidx, mc), scale=SCL)
                return ev

            def evict_add_comb(bidx):
                def ev(mc, rh, ps):
                    e = epool.tile([P, RH], F32, tag="e")
                    nc.scalar.activation(
                        e[:], ps[:], mybir.ActivationFunctionType.Relu,
                        bias=bias_ap(bidx, mc), scale=SCL)
                    nc.vector.tensor_add(comb[:, mc, rsl(rh)],
                                         e[:], comb[:, mc, rsl(rh)])
                return ev

            # ---- A = relu(hxn @ w1pre' + 0.7*b1pre), cached for all steps.
            # t0-n1 (snew = A + c1) is fused into the same pass so its
            # elementwise work overlaps the A matmuls chunk by chunk.
            red = red_pair()

            def ev_a(mc, rh, ps, red=red):
                nc.scalar.activation(
                    At[:, mc, rsl(rh)], ps[:],
                    mybir.ActivationFunctionType.Relu,
                    bias=bias_ap(B1PRE, mc), scale=SCL)
                nc.vector.tensor_scalar_add(
                    snew[:, mc, rsl(rh)], At[:, mc, rsl(rh)],
                    bias_ap(C1, mc))
                sq_and_reduce(mc, rh, red)

            # defer=4: the A pass produces chunks every ~0.5us (4 dr-chunks),
            # so the ~1.5us eviction chain needs extra slack to stay hidden
            term_pass("w1pre", KC1, hx, ev_a, w0_tile=w0, defer=4)
            finale(red, s1, None)

            # ---- t0, n2 / n3: single pre-term + const.
            # t1-n1's post/self term passes are wedged between them: they
            # only need s2(t0)/s1(t0) and don't touch comb (the t0 updates
            # don't use it), so their matmuls fill t0's serial-chain tails.
            def ev_t0(red, cidx, bpre):
                def ev(mc, rh, ps):
                    e = epool.tile([P, RH], F32, tag="e")
                    nc.scalar.activation(
                        e[:], ps[:], mybir.ActivationFunctionType.Relu,
                        bias=bias_ap(bpre, mc), scale=SCL)
                    nc.vector.tensor_scalar_add(
                        snew[:, mc, rsl(rh)], e[:], bias_ap(cidx, mc))
                    sq_and_reduce(mc, rh, red)
                return ev

            red = red_pair()
            term_pass("w2pre", KC, s1, ev_t0(red, C2, B2PRE))
            finale(red, s2, None)

            term_pass("w1post", KC, s2, evict_to(comb, B1POST))
            term_pass("w1self", KC, s1, evict_add_comb(B1SELF))

            red = red_pair()
            term_pass("w3pre", KC, s2, ev_t0(red, C3, B3PRE))
            finale(red, s3, None)

            def n1_chunk(red):
                """Per-mc hook: n1 = A + comb (640-wide DVE add + sq/reduce),
                interleaved into the following pass's matmul stream so the
                PE never drains while DVE churns through the combine."""
                def hook(mc):
                    nc.vector.tensor_add(snew[:, mc, :], At[:, mc, :],
                                         comb[:, mc, :])
                    sq_and_reduce(mc, 1, red)
                return hook

            # ---- t1 / t2
            for t in (1, 2):
                last = (t == 2)
                # n1 = A + relu(s2@w1post'+b) + relu(s1@w1self'+b)
                if t == 2:
                    term_pass("w1post", KC, s2, evict_to(comb, B1POST))
                    term_pass("w1self", KC, s1, evict_add_comb(B1SELF))

                # n2 = relu(s1new@w2pre') + relu(s3@w2post') + relu(s2@w2self')
                # n1's combine rides inside the w2post pass: chunk mc of the
                # combine is emitted just before w2post's mc matmul block
                # (the w2post eviction's comb[mc] write lands after the
                # combine's read, chunk by chunk)
                red_n1 = red_pair()
                term_pass("w2post", KC, s3, evict_to(comb, B2POST),
                          pre_chunk=n1_chunk(red_n1))
                finale(red_n1, s1, "init" if last else None)
                term_pass("w2self", KC, s2, evict_add_comb(B2SELF))
                red = red_pair()

                def ev_n2(mc, rh, ps, red=red):
                    e = epool.tile([P, RH], F32, tag="e")
                    nc.scalar.activation(
                        e[:], ps[:], mybir.ActivationFunctionType.Relu,
                        bias=bias_ap(B2PRE, mc), scale=SCL)
                    nc.vector.tensor_add(snew[:, mc, rsl(rh)],
                                         e[:], comb[:, mc, rsl(rh)])
                    sq_and_reduce(mc, rh, red)

                term_pass("w2pre", KC, s1, ev_n2)
                finale(red, s2, "add" if last else None)

                # n3 = relu(s2new@w3pre') + c3p + relu(s3@w3self')
                term_pass("w3self", KC, s3, evict_to(comb, B3SELF))
                red = red_pair()

                def ev_n3(mc, rh, ps, red=red):
                    e = epool.tile([P, RH], F32, tag="e")
                    nc.scalar.activation(
                        e[:], ps[:], mybir.ActivationFunctionType.Relu,
                        bias=bias_ap(B3PRE, mc), scale=SCL)
                    nc.vector.scalar_tensor_tensor(
                        snew[:, mc, rsl(rh)], e[:], bias_ap(C3P, mc),
                        comb[:, mc, rsl(rh)],
                        op0=mybir.AluOpType.add, op1=mybir.AluOpType.add)
                    sq_and_reduce(mc, rh, red)

                term_pass("w3pre", KC, s2, ev_n3)
                finale(red, None if last else s3, "add" if last else None)

            # ---- goodness out: g = gacc / (alpha^2 * 2048)
            gout = consts.tile([1, R], F32, tag="gout")
            nc.scalar.mul(gout[:], gacc[:], 1.0 / (AL * AL * H))
            nc.sync.dma_start(out=g_d[:], in_=gout[:])

    nc.compile()
    return nc


def _block_weight(w, scale, kcn):
    """[2048, d_in] float32 -> [MC, P, kcn, P] fp8e4m3 blocked for linear DMA:
    host_w[mc, p, kc, m] = scale * SW * W[mc*128+m, kc*128+p]."""
    w = np.asarray(w, dtype=np.float32) * (scale * SW)
    din = w.shape[1]
    if din < kcn * P:
        w = np.pad(w, ((0, 0), (0, kcn * P - din)))
    blk = w.reshape(MC, P, kcn, P).transpose(0, 3, 2, 1)
    return np.ascontiguousarray(blk.astype(NPF8))


def _col(v):
    """[2048] -> [128, 16] (partition-major bias layout)."""
    return np.asarray(v, dtype=np.float32).reshape(MC, P).T


def prepare_inputs(inputs):
    """Host prep: overlay+normalize Hx, prescale/block weights, pack biases.
    Returns (shared_map, per_core_hx list)."""
    x = np.asarray(inputs["x"], dtype=np.float32)
    mx = x.max()
    base = x.copy()
    base[:, :NL] = 0.0
    hx = np.tile(base[None, :, :], (NL, 1, 1))
    for l in range(NL):
        hx[l, :, l] = mx
    hx = hx.reshape(ROWS, D_IN)
    n = np.linalg.norm(hx, axis=1, keepdims=True)
    hxn = hx / (n + EPS) * SX
    hxn = np.pad(hxn, ((0, 0), (0, D_IN_PAD - D_IN)))

    per_core_hx = []
    for c in range(N_CORES):
        h = hxn[c * R:(c + 1) * R].T            # [1024, 640]
        h = h.reshape(KC1, P, R).transpose(1, 0, 2)
        per_core_hx.append(np.ascontiguousarray(h.astype(NPF8)))

    shared = {
        "w1pre": _block_weight(inputs["w1_pre"], 0.7, KC1),
        "w1post": _block_weight(inputs["w1_post"], 0.7, KC),
        "w1self": _block_weight(inputs["w1_self"], 0.3, KC),
        "w2pre": _block_weight(inputs["w2_pre"], 0.7, KC),
        "w2post": _block_weight(inputs["w2_post"], 0.7, KC),
        "w2self": _block_weight(inputs["w2_self"], 0.3, KC),
        "w3pre": _block_weight(inputs["w3_pre"], 0.7, KC),
        "w3self": _block_weight(inputs["w3_self"], 0.3, KC),
    }

    relu = lambda a: np.maximum(np.asarray(a, dtype=np.float32), 0.0)

    cols = np.empty((P, NBIAS * MC), dtype=np.float32)
    # all bias/const columns carry the alpha=16 state scaling (relu positive
    # homogeneity: AL*relu(u+b) = relu(AL*u + AL*b))
    vals = {
        B1PRE: AL * 0.7 * np.asarray(inputs["b1_pre"], np.float32),
        B1POST: AL * 0.7 * np.asarray(inputs["b1_post"], np.float32),
        B1SELF: AL * 0.3 * np.asarray(inputs["b1_self"], np.float32),
        B2PRE: AL * 0.7 * np.asarray(inputs["b2_pre"], np.float32),
        B2POST: AL * 0.7 * np.asarray(inputs["b2_post"], np.float32),
        B2SELF: AL * 0.3 * np.asarray(inputs["b2_self"], np.float32),
        B3PRE: AL * 0.7 * np.asarray(inputs["b3_pre"], np.float32),
        B3SELF: AL * 0.3 * np.asarray(inputs["b3_self"], np.float32),
        C1: AL * (0.7 * relu(inputs["b1_post"]) + 0.3 * relu(inputs["b1_self"])),
        C2: AL * (0.7 * relu(inputs["b2_post"]) + 0.3 * relu(inputs["b2_self"])),
        C3: AL * (0.7 * relu(inputs["b3_post"]) + 0.3 * relu(inputs["b3_self"])),
        C3P: AL * 0.7 * relu(inputs["b3_post"]),
    }
    for idx, v in vals.items():
        cols[:, idx * MC:(idx + 1) * MC] = _col(v)
    shared["biases"] = np.ascontiguousarray(cols)

    return shared, per_core_hx


def run(inputs, trace=False):
    shared, per_core_hx = prepare_inputs(inputs)
    if "nc" not in _NC_CACHE:
        _NC_CACHE["nc"] = _build_nc()
    nc = _NC_CACHE["nc"]
    in_maps = [dict(shared, hxn=per_core_hx[c]) for c in range(N_CORES)]
    res = run_bass_kernel_spmd(nc, in_maps, core_ids=list(range(N_CORES)),
                               trace=trace)
    g = np.concatenate([res.results[c]["g"][0] for c in range(N_CORES)])
    out = g.reshape(NL, B).T.astype(np.float32)
    return np.ascontiguousarray(out), res


def kernel(**inputs):
    out, _ = run(inputs, trace=False)
    return out
